# revision 26
# baseline (speedup 1.0000x reference)
"""EquivariantAttention kernel for 8 trn2 NeuronCores (Bass/Tile).

Strategy: shard edges by destination node (host sorts edges by dst).
Core c owns nodes [1250c, 1250(c+1)) and all edges pointing into them, so
edge-softmax and the scatter-sum are core-local (no collectives).

v2: bf16 everywhere + work spread across DVE/Pool/ACT/PE.
  - PE (bf16 matmuls): MLP1, MLP2 (rw), one-hot segment-sum accumulation.
  - ACT: relu (PSUM->SBUF bf16), rw PSUM->SBUF bf16 cast, LeakyRelu, Exp.
  - Pool (gpsimd): tmp = fe*basis products + d1-reduce, tree levels 3/4,
    scores products/reduce, wv mul (scalar_tensor_tensor is cheaper than
    tensor_tensor on Pool).
  - DVE: pc products (bf16 2x mode), tree levels 1/2/5, one-hot is_equal
    (tensor_scalar 4x mode), window flush.
  Softmax max-subtraction is skipped (scores bounded; exact-arith identical)
  and the per-edge division is folded into one per-node reciprocal:
    out[n] = segsum(ex*v)[n] / segsum(ex)[n].
  Segment sums via one-hot matmuls accumulated in PSUM per 128-node window.
"""

import time

import numpy as np

import concourse.bacc as bacc
import concourse.bass as bass
import concourse.mybir as mybir
import concourse.tile as tile
from concourse.bass_utils import run_bass_kernel_spmd

F32 = mybir.dt.float32
F32R = mybir.dt.float32r
F16 = mybir.dt.float16
AF = mybir.ActivationFunctionType
ALU = mybir.AluOpType

E = 160000
N = 10000
NC = 8
NPC = N // NC          # 1250 nodes per core
WIN = 128
NWIN = (NPC + WIN - 1) // WIN   # 10 windows per core
M1, M2, D1, D2, NREPS = 16, 8, 3, 3, 2
EDGE_DIM, HID, NHEADS = 32, 64, 4
HIDDEN = M2 * D2        # 24
HEAD = HIDDEN // NHEADS  # 6
TEMP = float(HIDDEN) ** (-0.5)

_CACHE = {}
LAST_RUN_S = None


def _build(T, toff):
    """Build the Bass program for T 128-edge tiles; toff[w] = first tile of
    window w (length NWIN+1)."""
    nc = bacc.Bacc(None, target_bir_lowering=False, debug=False)
    EP = T * 128
    ef_d = nc.dram_tensor("efT", [EDGE_DIM + 1, EP], F32R, kind="ExternalInput")
    pk_d = nc.dram_tensor("packed", [T, 128, 577], F32, kind="ExternalInput")
    w1_d = nc.dram_tensor("w1", [EDGE_DIM + 1, HID], F32R, kind="ExternalInput")
    w2_d = nc.dram_tensor("w2", [HID + 1, 768], F32R, kind="ExternalInput")
    io_d = nc.dram_tensor("iota", [128, 128], F32, kind="ExternalInput")
    out_d = nc.dram_tensor("out", [NWIN * 128, HIDDEN], F32, kind="ExternalOutput")

    with tile.TileContext(nc) as tc:
        with (
            tc.tile_pool(name="const", bufs=1) as cp,
            tc.tile_pool(name="sb", bufs=5) as pool,
            tc.tile_pool(name="ps", bufs=2, space="PSUM") as pp,
            tc.tile_pool(name="seg", bufs=2, space="PSUM") as sp,
        ):
            w1_sb = cp.tile([EDGE_DIM + 1, HID], F32R)
            nc.sync.dma_start(w1_sb[:], w1_d[:])
            w2_sb = cp.tile([HID + 1, 768], F32R)
            nc.sync.dma_start(w2_sb[:], w2_d[:])
            io_sb = cp.tile([128, 128], F32)
            nc.sync.dma_start(io_sb[:], io_d[:])
            # manual 3-deep rotation for h so the ones-row is set once
            h_bufs = [cp.tile([HID + 1, 128], F32R, name=f"hbuf{i}") for i in range(5)]
            for hb in h_bufs:
                nc.vector.memset(hb[HID : HID + 1, :].bitcast(F32), 1.0)

            for w in range(NWIN):
                seg = sp.tile([128, 28], F32, tag="seg")
                t0, t1 = toff[w], toff[w + 1]
                for t in range(t0, t1):
                    ef_t = pool.tile([EDGE_DIM + 1, 128], F32R, tag="ef")
                    nc.sync.dma_start(ef_t[:], ef_d[:, t * 128 : (t + 1) * 128])
                    pk_t = pool.tile([128, 577], F32, tag="pk")
                    nc.sync.dma_start(pk_t[:], pk_d[t])

                    # --- tmp = fe . basis  (Pool) ---
                    # host pre-replicated, d1-major: both [d1, d, m1, r]
                    prod = pool.tile([128, 288], F32, tag="prod")
                    nc.gpsimd.tensor_mul(prod[:], pk_t[:, 0:288], pk_t[:, 288:576])
                    # reduce over d1 (3 contiguous 96-blocks) -> tmpT [dd, m]
                    ta = pool.tile([128, 96], F32, tag="ta")
                    nc.gpsimd.tensor_add(ta[:], prod[:, 0:96], prod[:, 96:192])
                    tmpT = pool.tile([128, 96], F16, tag="tmpT")
                    nc.gpsimd.tensor_add(tmpT[:], ta[:], prod[:, 192:288])

                    # one-hot over window node slots (DVE tensor_scalar 4x)
                    oh = pool.tile([128, 128], F32, tag="oh")
                    nc.gpsimd.tensor_scalar(
                        oh[:], io_sb[:], pk_t[:, 576:577], None, op0=ALU.is_equal
                    )

                    # --- MLP layer 1 (PE, bf16) + relu (ACT) ---
                    h_ps = pp.tile([HID, 128], F32, tag="hps")
                    nc.tensor.matmul(h_ps[:], w1_sb[:], ef_t[:], start=True, stop=True)
                    h_sb = h_bufs[t % 5]
                    nc.scalar.activation(h_sb[0:HID, :], h_ps[:], AF.Relu)

                    # --- MLP layer 2 (PE, bf16) -> rw [128e, 768] ---
                    rw_sb = pool.tile([128, 768], F16, tag="rwsb")
                    rw_lo = pp.tile([128, 384], F32, tag="rwlo")
                    nc.tensor.matmul(
                        rw_lo[:], h_sb[:], w2_sb[:, 0:384], start=True, stop=True
                    )
                    nc.scalar.copy(rw_sb[:, 0:384], rw_lo[:])
                    rw_hi = pp.tile([128, 384], F32, tag="rwhi")
                    nc.tensor.matmul(
                        rw_hi[:], h_sb[:], w2_sb[:, 384:768], start=True, stop=True
                    )
                    nc.scalar.copy(rw_sb[:, 384:768], rw_hi[:])

                    # --- conv products (DVE, bf16 2x): pc[c, dd, j=m] ---
                    rwv = (
                        rw_sb[:]
                        .rearrange("p (c j) -> p c j", j=32)
                        .unsqueeze(2)
                        .broadcast_to([128, 24, 3, 32])
                    )
                    tmv = (
                        tmpT[:]
                        .rearrange("p (d j) -> p d j", j=32)
                        .unsqueeze(1)
                        .broadcast_to([128, 24, 3, 32])
                    )
                    pc = pool.tile([128, 2304], F16, tag="pc")
                    nc.vector.tensor_mul(
                        pc[:].rearrange("p (c d j) -> p c d j", d=3, j=32), rwv, tmv
                    )

                    # --- tree reduce over m: 16,8 on DVE; 4,2 on Pool; 1 on DVE
                    l1 = pool.tile([128, 1152], F16, tag="l1")
                    v32 = pc[:].rearrange("p (g j) -> p g j", j=32)
                    nc.vector.tensor_add(
                        l1[:].rearrange("p (g j) -> p g j", j=16),
                        v32[:, :, 0:16],
                        v32[:, :, 16:32],
                    )
                    l2 = pool.tile([128, 576], F16, tag="l2")
                    v16 = l1[:].rearrange("p (g j) -> p g j", j=16)
                    nc.vector.tensor_add(
                        l2[:].rearrange("p (g j) -> p g j", j=8),
                        v16[:, :, 0:8],
                        v16[:, :, 8:16],
                    )
                    l3 = pool.tile([128, 288], F16, tag="l3")
                    v8 = l2[:].rearrange("p (g j) -> p g j", j=8)
                    nc.gpsimd.tensor_add(
                        l3[:].rearrange("p (g j) -> p g j", j=4),
                        v8[:, :, 0:4],
                        v8[:, :, 4:8],
                    )
                    l4 = pool.tile([128, 144], F16, tag="l4")
                    v4 = l3[:].rearrange("p (g j) -> p g j", j=4)
                    nc.vector.tensor_add(
                        l4[:].rearrange("p (g j) -> p g j", j=2),
                        v4[:, :, 0:2],
                        v4[:, :, 2:4],
                    )
                    conv_t = pool.tile([128, 72], F32, tag="conv")
                    v2 = l4[:].rearrange("p (g j) -> p g j", j=2)
                    nc.gpsimd.tensor_add(
                        conv_t[:].rearrange("p (g j) -> p g j", j=1),
                        v2[:, :, 0:1],
                        v2[:, :, 1:2],
                    )

                    # --- scores -> leaky relu -> exp  (temp folded into W2) ---
                    p4 = pool.tile([128, 24], F32, tag="p4")
                    nc.gpsimd.tensor_mul(p4[:], conv_t[:, 0:24], conv_t[:, 24:48])
                    s4 = pool.tile([128, 4], F32, tag="s4")
                    nc.vector.tensor_reduce(
                        s4[:],
                        p4[:].rearrange("p (h j) -> p h j", j=6),
                        axis=mybir.AxisListType.X,
                        op=ALU.add,
                    )
                    l4a = pool.tile([128, 4], F32, tag="l4a")
                    nc.vector.scalar_tensor_tensor(
                        l4a[:], s4[:], 0.2, s4[:], op0=ALU.mult, op1=ALU.max
                    )
                    x_t = pool.tile([128, 28], F32, tag="xt")
                    nc.scalar.activation(x_t[:, 0:4], l4a[:], AF.Exp)
                    exb = x_t[:, 0:4].unsqueeze(2).broadcast_to([128, 4, 6])
                    nc.gpsimd.tensor_mul(
                        x_t[:, 4:28].rearrange("p (h j) -> p h j", j=6),
                        conv_t[:, 48:72].rearrange("p (h j) -> p h j", j=6),
                        exb,
                    )

                    nc.tensor.matmul(
                        seg[:],
                        oh[:],
                        x_t[:],
                        start=(t == t0),
                        stop=(t == t1 - 1),
                        skip_group_check=True,
                    )

                # flush window: out = num / den
                den = pool.tile([128, 4], F32, tag="den")
                nc.vector.tensor_scalar_add(den[:], seg[:, 0:4], 1e-30)
                rcp = pool.tile([128, 4], F32, tag="rcp")
                nc.vector.reciprocal(rcp[:], den[:])
                outw = pool.tile([128, HIDDEN], F32, tag="outw")
                nc.vector.tensor_mul(
                    outw[:].rearrange("p (h j) -> p h j", j=6),
                    seg[:, 4:28].rearrange("p (h j) -> p h j", j=6),
                    rcp[:].unsqueeze(2).broadcast_to([128, 4, 6]),
                )
                nc.sync.dma_start(out_d[w * 128 : (w + 1) * 128, :], outw[:])
    nc.finalize()
    return nc


def _prep(src, dst, basis, edge_feats, f, W1, b1, W2, b2):
    src = np.asarray(src).astype(np.int64)
    dst = np.asarray(dst).astype(np.int64)
    basis = np.asarray(basis, dtype=np.float32)
    edge_feats = np.asarray(edge_feats, dtype=np.float32)
    f = np.asarray(f, dtype=np.float32)

    # node blocks of 128; bin-pack blocks into (core, window) slots so each
    # window's max tile count across cores is tight.
    NB = (N + WIN - 1) // WIN          # 79
    order = np.argsort(dst, kind="stable")
    ds = dst[order]
    cuts = np.searchsorted(ds, np.arange(0, NB * WIN + 1, WIN))
    cnt = cuts[1:] - cuts[:-1]         # edges per block
    tb = np.maximum(1, (cnt + 127) // 128)
    # phantom blocks (no nodes/edges) to fill NC*NWIN slots
    nslots = NC * NWIN
    tb_all = np.concatenate([tb, np.zeros(nslots - NB, dtype=tb.dtype)])
    blk_order = np.argsort(-tb_all, kind="stable")   # desc by tiles
    # group g = window, position = core;  slot (c,w) <- block blk_order[w*NC+c]
    tw = np.zeros(NWIN, dtype=np.int64)
    slot_block = np.full((NC, NWIN), -1, dtype=np.int64)
    for w in range(NWIN):
        grp = blk_order[w * NC : (w + 1) * NC]
        tw[w] = max(1, tb_all[grp].max())
        for c in range(NC):
            slot_block[c, w] = grp[c]
    toff = np.zeros(NWIN + 1, dtype=np.int64)
    toff[1:] = np.cumsum(tw)
    T = int(toff[-1])
    EP = T * 128

    # shared tensors
    s = np.ones(768, dtype=np.float32)
    s[: 16 * 32] = TEMP**0.5  # k and q blocks carry sqrt(temp) each
    w1_aug = np.concatenate(
        [np.asarray(W1, dtype=np.float32).T, np.asarray(b1, dtype=np.float32)[None, :]]
    )  # [33, 64]
    w2_aug = np.concatenate(
        [
            np.asarray(W2, dtype=np.float32).T * s[None, :],
            (np.asarray(b2, dtype=np.float32) * s)[None, :],
        ]
    )  # [65, 768]
    iota = np.broadcast_to(
        np.arange(128, dtype=np.float32)[None, :], (128, 128)
    ).copy()

    in_maps = []
    for c in range(NC):
        efT = np.zeros((EDGE_DIM + 1, EP), dtype=np.float32)
        packed = np.zeros((T, 128, 577), dtype=np.float32)
        packed[:, :, 576] = -1.0
        for w in range(NWIN):
            b = slot_block[c, w]
            if b >= NB:
                continue
            idx = order[cuts[b] : cuts[b + 1]]
            k = len(idx)
            if k == 0:
                continue
            base = toff[w] * 128
            efT[:EDGE_DIM, base : base + k] = edge_feats[idx].T
            efT[EDGE_DIM, base : base + k] = 1.0
            flat = packed.reshape(T * 128, 577)
            # fe [k, 16, 3] replicated, d1-major: [d1(3), d(3), m1(16), r(2)]
            fe = f[src[idx]]  # [k, 16, 3]
            fe_full = np.broadcast_to(
                fe.transpose(0, 2, 1)[:, :, None, :, None], (k, 3, 3, 16, 2)
            )
            flat[base : base + k, 0:288] = fe_full.reshape(k, 288)
            # basis [k, d1, (r, dd)] -> [d1, dd, r], replicated over m1
            bt = basis[idx].reshape(k, 3, 2, 3)  # (d1, r, dd)
            btt = bt.transpose(0, 1, 3, 2)  # (d1, dd, r)
            bas_full = np.broadcast_to(btt[:, :, :, None, :], (k, 3, 3, 16, 2))
            flat[base : base + k, 288:576] = bas_full.reshape(k, 288)
            flat[base : base + k, 576] = (dst[idx] - b * WIN).astype(np.float32)

        in_maps.append(
            {
                "efT": efT,
                "packed": packed,
                "w1": w1_aug,
                "w2": w2_aug,
                "iota": iota,
            }
        )
    return T, toff, slot_block, in_maps


def kernel(src, dst, basis, edge_feats, f, W1, b1, W2, b2):
    global LAST_RUN_S
    T, toff, slot_block, in_maps = _prep(
        src, dst, basis, edge_feats, f, W1, b1, W2, b2
    )
    key = (T, tuple(toff))
    if key not in _CACHE:
        _CACHE[key] = _build(T, toff)
    nc = _CACHE[key]
    t0 = time.time()
    import os

    trace = bool(os.environ.get("BASS_KTRACE"))
    res = run_bass_kernel_spmd(nc, in_maps, list(range(NC)), trace=trace)
    LAST_RUN_S = time.time() - t0
    global LAST_RESULTS
    LAST_RESULTS = res
    NB = (N + WIN - 1) // WIN
    full = np.zeros((N, HIDDEN), dtype=np.float32)
    for c in range(NC):
        out_c = np.asarray(res.results[c]["out"], dtype=np.float32)
        for w in range(NWIN):
            b = slot_block[c, w]
            if b >= NB:
                continue
            bs = min(WIN, N - b * WIN)
            full[b * WIN : b * WIN + bs] = out_c[w * 128 : w * 128 + bs]
    return full.reshape(N, M2, D2)
